# revision 4
# baseline (speedup 1.0000x reference)
"""W8A16 column-parallel linear for TRN2, 8 NeuronCores.

Computes y = x @ (qweight * w_scales).T + bias with
  x        [8, 1, 8192]  fp16
  qweight  [28672, 8192] int8 (per-row symmetric quant)
  w_scales [28672, 1]    fp16
  bias     [28672]       fp16
  y        [8, 1, 28672] fp16

Sharding: column-parallel - each of the 8 cores owns 3584 output rows
(qweight/w_scales/bias shard), x replicated. No collectives; outputs are
concatenated on the host.

Per-core kernel, span-major streaming: the 3584 output columns are cut
into 7 spans of 512. The int8 weight shard streams span-by-span from HBM
(host pre-arranges each DMA group as a contiguous [128, u*512] slab so
descriptors are u*512 B per partition), is converted int8->fp16 on-chip
(k-split between VectorE 2x-port mode and ScalarE), and accumulates into
that span's PSUM bank with fp16 matmuls (stationary x^T tile, moving
weight tile). Each span is split into 2x256-col halves on different PE
column groups so even a HAM-cold PE outruns the DMA pace. When a span's
64 k-tiles finish, its scale-multiply (out=(sum x*q + b/s)*s) and output
DMA run *under* the next span's weight stream - unlike the k-major
baseline whose 3 serial full-width scale-muls + output DMAs formed an
~18 us tail after the last weight byte. The last span tapers its group
sizes so the drain after the final weight byte is short.

Baseline (k-major): ~106 us. HBM floor: 29.36 MB int8 / 358 GB/s ~ 82 us
per core, + ~7 us NEFF preamble before the first DMA byte.
"""

import numpy as np

import concourse.bacc as bacc
import concourse.mybir as mybir
import concourse.tile as tile
from concourse.bass_utils import run_bass_kernel_spmd

B, S, K, N = 8, 1, 8192, 28672
M = B * S                 # 8 rows in the GEMM
NCORES = 8
NS = N // NCORES          # 3584 output rows per core
KT = K // 128             # 64 k-tiles
NSPAN = 7                 # spans of 512 output cols (one PSUM bank each)
SW = 512                  # span width

# group sizes (k-tiles per DMA/convert group) for spans 0..5 and span 6:
# uniform big groups keep DMA+conversion efficient; span 6 tapers so the
# pipeline drains quickly after the last weight byte lands.
GROUPS_MAIN = [16, 16, 16, 16]
GROUPS_LAST = [16, 16, 16, 8, 4, 2, 1, 1]
assert sum(GROUPS_MAIN) == KT and sum(GROUPS_LAST) == KT

# k-split of each group's int8->fp16 conversion: VectorE (2 elem/cyc)
# takes DVE_U[u] k-tiles, ScalarE (1 elem/cyc @1.2GHz) the rest.
DVE_U = {16: 10, 8: 5, 4: 3, 2: 2, 1: 1}

_CACHE = {}


def _span_groups(sp):
    return GROUPS_LAST if sp == NSPAN - 1 else GROUPS_MAIN


def _u_classes():
    """(u -> number of groups of that size across the whole kernel)."""
    cnt = {}
    for sp in range(NSPAN):
        for u in _span_groups(sp):
            cnt[u] = cnt.get(u, 0) + 1
    return cnt


def _build():
    nc = bacc.Bacc()
    xp = nc.declare_dram_parameter("x", [128, KT * M + M], mybir.dt.float16,
                                   isOutput=False)
    ucnt = _u_classes()
    qps = {
        u: nc.declare_dram_parameter(f"q{u}", [n, 128, u * SW], mybir.dt.int8,
                                     isOutput=False)
        for u, n in sorted(ucnt.items())
    }
    sp_ = nc.declare_dram_parameter("s", [M, NS], mybir.dt.float16, isOutput=False)
    bp = nc.declare_dram_parameter("b", [1, NS], mybir.dt.float16, isOutput=False)
    op = nc.declare_dram_parameter("out", [M, NS], mybir.dt.float16, isOutput=True)
    # whole-param rearranges: keep the (u n) free dim contiguous per
    # partition so each group DMA is 128 descriptors of u*512 bytes
    qv = {u: qps[u].rearrange("g p (u n) -> g p u n", u=u) for u in qps}

    with tile.TileContext(nc) as tc:
        with (
            tc.tile_pool(name="const", bufs=1) as constp,
            tc.tile_pool(name="wq", bufs=4) as wqp,
            tc.tile_pool(name="wf", bufs=3) as wfp,
            tc.tile_pool(name="psum", bufs=1, space="PSUM") as psp,
        ):
            xsb = constp.tile([128, KT * M + M], mybir.dt.float16, tag="xsb")
            sb = constp.tile([72, NS], mybir.dt.float16, tag="sb")
            b1 = constp.tile([1, NS], mybir.dt.float16, tag="b1")
            osb = constp.tile([72, NS], mybir.dt.float16, tag="osb")
            # ones row for the bias-opening matmuls lives in xsb's last
            # M columns (host packs 1.0 at partition 0 there)
            ones = xsb[0:1, KT * M:KT * M + M]

            psum = psp.tile([128, NS], mybir.dt.float32, tag="psum")

            # the weight stream is the binding resource: its first group
            # leads the HWDGE queue, constants ride behind it
            qidx = {u: 0 for u in ucnt}
            u0 = _span_groups(0)[0]
            wq0 = wqp.tile([128, u0, SW], mybir.dt.int8, tag="wq")
            nc.sync.dma_start(wq0[:], qv[u0][qidx[u0]])
            qidx[u0] += 1
            nc.sync.dma_start(xsb[:], xp[:])
            nc.sync.dma_start(b1[:], bp[:])
            for j in range(3):
                nc.sync.dma_start(sb[32 * j:32 * j + M, :], sp_[:])

            for sp in range(NSPAN):
                ga, gb = (2 * sp) % 3, (2 * sp + 1) % 3
                pa, pb = 32 * ga, 32 * gb
                ca, cb = sp * SW, sp * SW + SW // 2   # column starts of halves
                psA = psum[pa:pa + M, ca:ca + SW // 2]
                psB = psum[pb:pb + M, cb:cb + SW // 2]

                # bias rows open the accumulation: psum = ones^T @ (b/s)
                nc.tensor.matmul(psA, ones, b1[:, ca:ca + SW // 2],
                                 start=True, stop=False)
                nc.tensor.matmul(psB, ones, b1[:, cb:cb + SW // 2],
                                 start=True, stop=False)

                kt0 = 0
                for g, u in enumerate(_span_groups(sp)):
                    if sp == 0 and g == 0:
                        wq = wq0
                    else:
                        wq = wqp.tile([128, u, SW], mybir.dt.int8, tag="wq")
                        nc.sync.dma_start(wq[:], qv[u][qidx[u]])
                        qidx[u] += 1
                    wf = wfp.tile([128, u, SW], mybir.dt.float16, tag="wf")
                    u1 = DVE_U[u]
                    nc.vector.tensor_copy(wf[:, 0:u1, :], wq[:, 0:u1, :])
                    if u1 < u:
                        nc.scalar.activation(
                            wf[:, u1:u, :], wq[:, u1:u, :],
                            mybir.ActivationFunctionType.Copy,
                        )
                    for ui in range(u):
                        kt = kt0 + ui
                        last = kt == KT - 1
                        xt = xsb[:, kt * M:(kt + 1) * M]
                        nc.tensor.matmul(psA, xt, wf[:, ui, 0:SW // 2],
                                         start=False, stop=last)
                        nc.tensor.matmul(psB, xt, wf[:, ui, SW // 2:SW],
                                         start=False, stop=last)
                    kt0 += u

                # span epilogue: scale + output DMA; overlaps the next
                # span's weight stream
                nc.vector.tensor_mul(osb[pa:pa + M, ca:ca + SW // 2], psA,
                                     sb[pa:pa + M, ca:ca + SW // 2])
                nc.sync.dma_start(op[:, ca:ca + SW // 2],
                                  osb[pa:pa + M, ca:ca + SW // 2])
                nc.vector.tensor_mul(osb[pb:pb + M, cb:cb + SW // 2], psB,
                                     sb[pb:pb + M, cb:cb + SW // 2])
                nc.sync.dma_start(op[:, cb:cb + SW // 2],
                                  osb[pb:pb + M, cb:cb + SW // 2])

    nc.compile()
    return nc


def _get_nc():
    if "nc" not in _CACHE:
        _CACHE["nc"] = _build()
    return _CACHE["nc"]


def _prep_inputs(x, qweight, w_scales, bias):
    x2 = np.asarray(x, dtype=np.float16).reshape(M, K)
    # xsb[p, kt*M + m] = x[m, kt*128 + p]; last M cols: ones row at p=0
    xsb = np.zeros((128, KT * M + M), dtype=np.float16)
    xsb[:, :KT * M] = x2.T.reshape(KT, 128, M).transpose(1, 0, 2).reshape(128, KT * M)
    xsb[0, KT * M:] = 1.0
    qweight = np.asarray(qweight)
    w_scales = np.asarray(w_scales, dtype=np.float16).reshape(N)
    bias = np.asarray(bias, dtype=np.float16).reshape(N)
    ucnt = _u_classes()
    in_maps = []
    for c in range(NCORES):
        sl = slice(c * NS, (c + 1) * NS)
        # A[kt, p, sp, n] = q[sp*512+n, kt*128+p]
        A = np.ascontiguousarray(qweight[sl, :].T).reshape(KT, 128, NSPAN, SW)
        qarr = {u: np.empty((n, 128, u * SW), dtype=np.int8)
                for u, n in ucnt.items()}
        qidx = {u: 0 for u in ucnt}
        for sp in range(NSPAN):
            kt0 = 0
            for u in _span_groups(sp):
                blk = A[kt0:kt0 + u, :, sp, :].transpose(1, 0, 2)  # [128,u,SW]
                qarr[u][qidx[u]] = blk.reshape(128, u * SW)
                qidx[u] += 1
                kt0 += u
        srep = np.broadcast_to(w_scales[sl], (M, NS)).astype(np.float16)
        # bias enters the PSUM accumulation before the scale multiply:
        # out = (sum x*q + b/s) * s
        bos = (bias[sl].astype(np.float32)
               / w_scales[sl].astype(np.float32)).astype(np.float16)
        im = {"x": xsb, "s": np.ascontiguousarray(srep),
              "b": np.ascontiguousarray(bos.reshape(1, NS))}
        for u in ucnt:
            im[f"q{u}"] = qarr[u]
        in_maps.append(im)
    return in_maps


def _run(x, qweight, w_scales, bias, trace=False):
    nc = _get_nc()
    in_maps = _prep_inputs(x, qweight, w_scales, bias)
    res = run_bass_kernel_spmd(
        nc, in_maps, core_ids=list(range(NCORES)), trace=trace
    )
    y = np.concatenate(
        [np.asarray(res.results[c]["out"]) for c in range(NCORES)], axis=1
    )
    return y.reshape(B, S, N).astype(np.float16), res


def kernel(x, qweight, w_scales, bias):
    y, _ = _run(x, qweight, w_scales, bias, trace=False)
    return y


def kernel_traced(x, qweight, w_scales, bias):
    """Like kernel() but also returns the BassKernelResults (exec_time_ns)."""
    return _run(x, qweight, w_scales, bias, trace=True)
